# revision 7
# baseline (speedup 1.0000x reference)
"""Trainium2 Bass kernel for nn_ABNet: 10-head MLP + CBF-QP projection.

Data-parallel over 8 NeuronCores: batch 32768 -> 4096 rows/core, weights
replicated. MLP matmuls run in bf16 on the TensorEngine with activations in
[feature, batch] layout; the per-sample CBF geometry + QP halfspace
projection runs in fp32 on Vector/Scalar engines in batch-major layout.
"""

import sys

for _p in ("/opt/trn_rl_repo", "/root/.axon_site/_ro/trn_rl_repo"):
    if _p not in sys.path:
        sys.path.append(_p)

import numpy as np
import ml_dtypes

import concourse.bass as bass
import concourse.mybir as mybir
from concourse.tile import TileContext
from concourse.masks import make_identity

H = 10
B = 32768
NCORES = 8
BC = B // NCORES          # batch per core = 4096
NCHUNK = 8                # batch chunks per core
CH = BC // NCHUNK         # chunk width = 512
NT = BC // 128            # 128-wide batch tiles = 32
D = 256
L1C, L2C = 3.0, 3.0
OBS_X, OBS_Y, R = 0.0, 7.0, 4.0

F32 = mybir.dt.float32
BF16 = mybir.dt.bfloat16
AF = mybir.ActivationFunctionType
OP = mybir.AluOpType

MAX_WAITS = 1


def _split_waits(nc):
    """walrus in this env rejects >1 sync-wait per instruction; split extras
    onto preceding same-engine NOPs (semantically identical: sequential
    waits on an in-order engine)."""
    fn = nc.m.functions[0]
    for bb in fn.blocks:
        new = []
        for inst in bb.instructions:
            si = getattr(inst, "sync_info", None)
            if si is not None and si.on_wait and len(si.on_wait) > MAX_WAITS:
                waits = list(si.on_wait)
                while len(waits) > MAX_WAITS:
                    chunk, waits = waits[:MAX_WAITS], waits[MAX_WAITS:]
                    new.append(
                        mybir.InstNoOp(
                            name=nc.get_next_instruction_name(),
                            engine=inst.engine,
                            sync_info=mybir.SyncInfo(on_wait=chunk, on_update=[]),
                            bass_nofuse=True,
                        )
                    )
                si.on_wait = waits
            new.append(inst)
        bb.instructions[:] = new


def _bcast(t_ap, n=128):
    """Partition-broadcast AP for a DRAM tensor (step-0 partition dim)."""
    return bass.AP(tensor=t_ap.tensor, offset=t_ap.offset, ap=[[0, n]] + list(t_ap.ap))


def build_graph():
    nc = bass.Bass()
    dp = nc.declare_dram_parameter

    xT = dp("xT", [4, BC], BF16, isOutput=False)
    xg = dp("xg", [128, NT, 4], F32, isOutput=False)
    w1t = dp("w1t", [4, H * D], BF16, isOutput=False)
    wmid = {
        name: dp(name, [H, 128, 2, D], BF16, isOutput=False)
        for name in ("w2", "w31", "w32", "w41", "w42")
    }
    bmid = {
        name: dp(name, [128, H, 2], F32, isOutput=False)
        for name in ("b1", "b2", "b31", "b32", "b41", "b42")
    }
    l5a = dp("l5a", [H, 2, 128, 4], BF16, isOutput=False)  # -W51T in cols 0:2
    l5b = dp("l5b", [H, 2, 128, 4], BF16, isOutput=False)  # +W52T in cols 2:4
    b51v = dp("b51v", [1, 2 * H], F32, isOutput=False)
    b52v = dp("b52v", [1, 2 * H], F32, isOutput=False)
    wt = dp("wt", [1, H], F32, isOutput=False)
    mlab = dp("mlab", [1, 2], F32, isOutput=False)
    slab = dp("slab", [1, 2], F32, isOutput=False)
    meanp = dp("meanp", [1, 4], F32, isOutput=False)
    stdp = dp("stdp", [1, 4], F32, isOutput=False)
    out = dp("out", [BC, 2], F32, isOutput=True)

    wsm_dram = nc.dram_tensor("wsm_dram", [1, H], F32)

    with TileContext(nc) as tc:
        wpool = tc.alloc_tile_pool(name="weights", bufs=1)
        apool = tc.alloc_tile_pool(name="acts", bufs=1)
        qpool = tc.alloc_tile_pool(name="qp", bufs=1)
        ppool = tc.alloc_tile_pool(name="psum", bufs=4, space="PSUM")
        p5pool = tc.alloc_tile_pool(name="psum5", bufs=2, space="PSUM")
        ptpool = tc.alloc_tile_pool(name="psumT", bufs=2, space="PSUM")

        # ---- weight / input loads -------------------------------------
        xT_sb = wpool.tile([4, BC], BF16)
        nc.sync.dma_start(out=xT_sb, in_=xT[:, :])
        xg_sb = qpool.tile([128, NT, 4], F32)
        nc.sync.dma_start(out=xg_sb, in_=xg[:, :, :])

        # small tensors first so the first head's evacuations aren't queued
        # behind megabytes of weight DMAs
        bmid_sb = {}
        for name, t in bmid.items():
            btile = wpool.tile([128, H, 2], F32, name=f"{name}_sb")
            nc.sync.dma_start(out=btile, in_=t[:, :, :])
            bmid_sb[name] = btile
        w1_sb = wpool.tile([4, H * D], BF16)
        nc.sync.dma_start(out=w1_sb, in_=w1t[:, :])
        l5a_sb = []
        l5b_sb = []
        for h in range(H):
            ta = wpool.tile([128, 2, 4], BF16, name=f"l5a_{h}")
            nc.sync.dma_start(out=ta[:, 0, :], in_=l5a[h, 0])
            nc.sync.dma_start(out=ta[:, 1, :], in_=l5a[h, 1])
            l5a_sb.append(ta)
            tb = wpool.tile([128, 2, 4], BF16, name=f"l5b_{h}")
            nc.sync.dma_start(out=tb[:, 0, :], in_=l5b[h, 0])
            nc.sync.dma_start(out=tb[:, 1, :], in_=l5b[h, 1])
            l5b_sb.append(tb)
        # per-head weight order matches consumption order
        wmid_sb = {name: [None] * H for name in wmid}
        for h in range(H):
            for name in ("w2", "w31", "w32", "w41", "w42"):
                wtile = wpool.tile([128, 2, D], BF16, name=f"{name}_{h}")
                nc.sync.dma_start(out=wtile, in_=wmid[name][h])
                wmid_sb[name][h] = wtile

        # broadcasts ([128, n] copies of small vectors)
        def bc_tile(src_ap, n, name):
            t = qpool.tile([128, n], F32, name=name)
            nc.sync.dma_start(out=t, in_=_bcast(src_ap))
            return t

        b51_bc = bc_tile(b51v[0], 2 * H, "b51bc")
        b52_bc = bc_tile(b52v[0], 2 * H, "b52bc")
        mlab_bc = bc_tile(mlab[0], 2, "mlabbc")
        slab_bc = bc_tile(slab[0], 2, "slabbc")
        mean_bc = bc_tile(meanp[0], 4, "meanbc")
        std_bc = bc_tile(stdp[0], 4, "stdbc")

        ident = wpool.tile([128, 128], F32, name="ident")
        make_identity(nc, ident[:, :])

        # ---- softmax(wt) on device ------------------------------------
        wt_sb = qpool.tile([1, H], F32, name="wt_sb")
        nc.sync.dma_start(out=wt_sb, in_=wt[:, :])
        wexp = qpool.tile([1, H], F32, name="wexp")
        zeroh = qpool.tile([1, 1], F32, name="zeroh")
        nc.vector.memset(zeroh, 0.0)
        nc.scalar.activation(wexp, wt_sb, AF.Exp, bias=zeroh[:, 0:1])
        wsum = qpool.tile([1, 1], F32, name="wsum")
        nc.vector.reduce_sum(out=wsum, in_=wexp, axis=mybir.AxisListType.X)
        winv = qpool.tile([1, 1], F32, name="winv")
        nc.vector.reciprocal(winv, wsum)
        wnorm = qpool.tile([1, H], F32, name="wnorm")
        nc.vector.tensor_scalar(wnorm, wexp, winv[:, 0:1], None, OP.mult)
        nc.sync.dma_start(out=wsm_dram[:, :], in_=wnorm)
        w_bc = bc_tile(wsm_dram[0], H, "wbc")

        # ---- geometry (batch-major [128, NT] fp32) --------------------
        def qt(name):
            return qpool.tile([128, NT], F32, name=name)

        tt = nc.vector.tensor_tensor
        ts = nc.vector.tensor_scalar
        stt = nc.vector.scalar_tensor_tensor

        # de-normalized state rows
        t1q, w1q, t2q, w2q = qt("t1q"), qt("w1q"), qt("t2q"), qt("w2q")
        for dst, f in ((t1q, 0), (w1q, 1), (t2q, 2), (w2q, 3)):
            ts(dst, xg_sb[:, :, f], std_bc[:, f : f + 1], mean_bc[:, f : f + 1],
               OP.mult, OP.add)

        s1, c1, s2, c2 = qt("s1"), qt("c1"), qt("s2"), qt("c2")
        zero1 = qpool.tile([128, 1], F32, name="zero1")
        nc.vector.memset(zero1, 0.0)
        pi2 = qpool.tile([128, 1], F32, name="pi2")
        nc.vector.memset(pi2, float(np.pi / 2))
        nc.scalar.activation(s1, t1q, AF.Sin, bias=zero1[:, 0:1])
        nc.scalar.activation(c1, t1q, AF.Sin, bias=pi2[:, 0:1])
        nc.scalar.activation(s2, t2q, AF.Sin, bias=zero1[:, 0:1])
        nc.scalar.activation(c2, t2q, AF.Sin, bias=pi2[:, 0:1])

        px, py, vx, vy = qt("px"), qt("py"), qt("vx"), qt("vy")
        tmp1, tmp2, tmp3 = qt("tmp1"), qt("tmp2"), qt("tmp3")
        tt(tmp1, c1, c2, OP.add)
        ts(px, tmp1, L1C, -OBS_X, OP.mult, OP.add)
        tt(tmp1, s1, s2, OP.add)
        ts(py, tmp1, L1C, -OBS_Y, OP.mult, OP.add)
        tt(tmp1, s1, w1q, OP.mult)
        tt(tmp2, s2, w2q, OP.mult)
        tt(tmp3, tmp1, tmp2, OP.add)
        ts(vx, tmp3, -L1C, None, OP.mult)
        tt(tmp1, c1, w1q, OP.mult)
        tt(tmp2, c2, w2q, OP.mult)
        tt(tmp3, tmp1, tmp2, OP.add)
        ts(vy, tmp3, L1C, None, OP.mult)

        bar16, bdot4, lf2b = qt("bar16"), qt("bdot4"), qt("lf2b")
        tt(tmp1, px, px, OP.mult)
        tt(tmp2, py, py, OP.mult)
        tt(tmp3, tmp1, tmp2, OP.add)
        ts(bar16, tmp3, 16.0, -16.0 * R * R, OP.mult, OP.add)
        tt(tmp1, px, vx, OP.mult)
        tt(tmp2, py, vy, OP.mult)
        tt(tmp3, tmp1, tmp2, OP.add)
        ts(bdot4, tmp3, 8.0, None, OP.mult)

        w1sq, w2sq = qt("w1sq"), qt("w2sq")
        tt(w1sq, w1q, w1q, OP.mult)
        tt(w2sq, w2q, w2q, OP.mult)
        # lf2b = 2(vx^2 + vy^2) - 6*[px*(c1 w1sq + c2 w2sq) + py*(s1 w1sq + s2 w2sq)]
        tt(tmp1, c1, w1sq, OP.mult)
        tt(tmp2, c2, w2sq, OP.mult)
        tt(tmp3, tmp1, tmp2, OP.add)
        ua = qt("ua")
        tt(ua, px, tmp3, OP.mult)
        tt(tmp1, s1, w1sq, OP.mult)
        tt(tmp2, s2, w2sq, OP.mult)
        tt(tmp3, tmp1, tmp2, OP.add)
        ub = qt("ub")
        tt(ub, py, tmp3, OP.mult)
        tt(tmp1, ua, ub, OP.add)
        tt(tmp2, vx, vx, OP.mult)
        tt(tmp3, vy, vy, OP.mult)
        ud = qt("ud")
        tt(ud, tmp2, tmp3, OP.add)
        # lf2b = 2*ud - 6*tmp1  ->  (ud*2) add (tmp1*-6): two-step
        ts(tmp2, tmp1, -6.0, None, OP.mult)
        stt(lf2b, ud, 2.0, tmp2, OP.mult, OP.add)

        g1, g2, igg = qt("g1"), qt("g2"), qt("igg")
        tt(tmp1, px, s1, OP.mult)
        tt(tmp2, py, c1, OP.mult)
        tt(tmp3, tmp1, tmp2, OP.subtract)
        ts(g1, tmp3, 2.0 * L1C, None, OP.mult)
        tt(tmp1, px, s2, OP.mult)
        tt(tmp2, py, c2, OP.mult)
        tt(tmp3, tmp1, tmp2, OP.subtract)
        ts(g2, tmp3, 2.0 * L2C, None, OP.mult)
        tt(tmp1, g1, g1, OP.mult)
        tt(tmp2, g2, g2, OP.mult)
        tt(tmp3, tmp1, tmp2, OP.add)
        nc.vector.reciprocal(igg, tmp3)

        # ---- MLP head loop --------------------------------------------
        x5all = qpool.tile([4 * H, BC], F32, name="x5all")

        def evac(engine_idx, dst_ap, psum_ap, bias_ap):
            """relu(psum + bias) -> bf16 dst; engine picked by parity."""
            if engine_idx % 2 == 0:
                nc.scalar.activation(dst_ap, psum_ap, AF.Relu, bias=bias_ap)
            else:
                nc.vector.tensor_scalar(dst_ap, psum_ap, bias_ap, 0.0, OP.add, OP.max)

        for h in range(H):
            a1 = apool.tile([128, 2, BC], BF16, tag="t_x1", name=f"x1_{h}")
            a2 = apool.tile([128, 2, BC], BF16, tag="t_x2", name=f"x2_{h}")
            a31 = apool.tile([128, 2, BC], BF16, tag="t_x31", name=f"x31_{h}")
            a32 = apool.tile([128, 2, BC], BF16, tag="t_x32", name=f"x32_{h}")
            a41 = apool.tile([128, 2, BC], BF16, tag="t_x41", name=f"x41_{h}")
            a42 = apool.tile([128, 2, BC], BF16, tag="t_x42", name=f"x42_{h}")

            # L1: K=4
            for mt in range(2):
                for c in range(NCHUNK):
                    ps = ppool.tile([128, CH], F32)
                    nc.tensor.matmul(
                        ps,
                        w1_sb[:, h * D + mt * 128 : h * D + (mt + 1) * 128],
                        xT_sb[:, c * CH : (c + 1) * CH],
                        start=True, stop=True,
                    )
                    evac(mt, a1[:, mt, c * CH : (c + 1) * CH], ps,
                         bmid_sb["b1"][:, h, mt : mt + 1])

            # mid layers
            for wname, bname, src, dst in (
                ("w2", "b2", a1, a2),
                ("w31", "b31", a2, a31),
                ("w32", "b32", a2, a32),
                ("w41", "b41", a31, a41),
                ("w42", "b42", a32, a42),
            ):
                wtiles = wmid_sb[wname][h]
                btile = bmid_sb[bname]
                for mt in range(2):
                    for c in range(NCHUNK):
                        ps = ppool.tile([128, CH], F32)
                        for kt in range(2):
                            nc.tensor.matmul(
                                ps,
                                wtiles[:, kt, mt * 128 : (mt + 1) * 128],
                                src[:, kt, c * CH : (c + 1) * CH],
                                start=(kt == 0), stop=(kt == 1),
                            )
                        evac(mt, dst[:, mt, c * CH : (c + 1) * CH], ps,
                             btile[:, h, mt : mt + 1])

            # L5: [-W51^T x41 | W52^T x42] -> psum [4, CH]
            for c in range(NCHUNK):
                ps5 = p5pool.tile([4, CH], F32)
                nc.tensor.matmul(ps5, l5a_sb[h][:, 0, :], a41[:, 0, c * CH : (c + 1) * CH],
                                 start=True, stop=False)
                nc.tensor.matmul(ps5, l5a_sb[h][:, 1, :], a41[:, 1, c * CH : (c + 1) * CH],
                                 start=False, stop=False)
                nc.tensor.matmul(ps5, l5b_sb[h][:, 0, :], a42[:, 0, c * CH : (c + 1) * CH],
                                 start=False, stop=False)
                nc.tensor.matmul(ps5, l5b_sb[h][:, 1, :], a42[:, 1, c * CH : (c + 1) * CH],
                                 start=False, stop=True)
                stg5 = apool.tile([4, CH], F32, tag="t_stg5", bufs=4)
                if c % 2 == 0:
                    nc.scalar.activation(stg5, ps5, AF.Copy)
                else:
                    nc.vector.tensor_copy(stg5, ps5)
                nc.sync.dma_start(
                    out=x5all[4 * h : 4 * h + 4, c * CH : (c + 1) * CH], in_=stg5
                )

        # ---- transpose x5all [40, BC] -> x5T [128, NT, 40] ------------
        x5T = qpool.tile([128, NT, 4 * H], F32, name="x5T")
        for t in range(NT):
            pst = ptpool.tile([128, 4 * H], F32)
            nc.tensor.transpose(
                pst, x5all[:, t * 128 : (t + 1) * 128], ident[0 : 4 * H, 0 : 4 * H]
            )
            nc.vector.tensor_copy(x5T[:, t, :], pst)

        # ---- QP projection --------------------------------------------
        # x5T cols per head h: 0:-(W51 x41) c0, 1: c1, 2: z52 c0, 3: z52 c1
        # u_unc = col - b51 ; sigma = sigmoid(z52 + b52)
        for h in range(H):
            for cix in range(2):
                ts(x5T[:, :, 4 * h + cix], x5T[:, :, 4 * h + cix],
                   b51_bc[:, 2 * h + cix : 2 * h + cix + 1], None, OP.subtract)
                nc.scalar.activation(
                    x5T[:, :, 4 * h + 2 + cix], x5T[:, :, 4 * h + 2 + cix],
                    AF.Sigmoid, bias=b52_bc[:, 2 * h + cix : 2 * h + cix + 1],
                )

        # shared per-sample terms using head-0's sigma_a
        s0 = x5T[:, :, 2]
        Aq, Dq = qt("Aq"), qt("Dq")
        tt(tmp1, s0, bdot4, OP.mult)
        tt(Aq, lf2b, tmp1, OP.add)
        tt(tmp1, s0, bar16, OP.mult)
        tt(Dq, bdot4, tmp1, OP.add)

        acc_e, acc_u1, acc_u2 = qt("acc_e"), qt("acc_u1"), qt("acc_u2")
        nc.vector.memset(acc_e, 0.0)
        nc.vector.memset(acc_u1, 0.0)
        nc.vector.memset(acc_u2, 0.0)

        hq, e_q = qt("hq"), qt("e_q")
        for h in range(H):
            u1 = x5T[:, :, 4 * h + 0]
            u2 = x5T[:, :, 4 * h + 1]
            sb_i = x5T[:, :, 4 * h + 3]
            # h_i = A + sb_i * D
            tt(tmp1, sb_i, Dq, OP.mult)
            tt(hq, Aq, tmp1, OP.add)
            # viol = u1*g1 + u2*g2 - h_i ; e = max(viol, 0)
            tt(tmp1, u1, g1, OP.mult)
            tt(tmp2, u2, g2, OP.mult)
            tt(tmp3, tmp1, tmp2, OP.add)
            stt(tmp1, hq, -1.0, tmp3, OP.mult, OP.add)
            ts(e_q, tmp1, 0.0, None, OP.max)
            # weighted accumulation
            wh = w_bc[:, h : h + 1]
            stt(acc_e, e_q, wh, acc_e, OP.mult, OP.add)
            stt(acc_u1, u1, wh, acc_u1, OP.mult, OP.add)
            stt(acc_u2, u2, wh, acc_u2, OP.mult, OP.add)

        # out_c = (acc_uc - acc_e*igg*g_c - mlab_c) / slab_c
        lamw = qt("lamw")
        tt(lamw, acc_e, igg, OP.mult)
        isl = qpool.tile([128, 2], F32, name="isl")
        nc.vector.reciprocal(isl, slab_bc)
        out_t = qpool.tile([128, NT, 2], F32, name="out_t")
        for cix, (accu, g_c) in enumerate(((acc_u1, g1), (acc_u2, g2))):
            tt(tmp1, lamw, g_c, OP.mult)
            tt(tmp2, accu, tmp1, OP.subtract)
            ts(out_t[:, :, cix], tmp2, mlab_bc[:, cix : cix + 1],
               isl[:, cix : cix + 1], OP.subtract, OP.mult)

        nc.sync.dma_start(
            out=out.rearrange("(t p) c -> p t c", p=128), in_=out_t
        )

        for pool in (ptpool, p5pool, ppool, qpool, apool, wpool):
            pool.release()

    _split_waits(nc)
    return nc


def prep_inputs(inputs):
    """Host-side shard + layout prep. Returns in_maps for 8 cores."""
    f32 = np.float32
    bf16 = ml_dtypes.bfloat16
    x = np.asarray(inputs["x"], f32)

    def wT(W):  # [H, dout, din] -> [H, din, dout]
        return np.ascontiguousarray(np.asarray(W, f32).transpose(0, 2, 1))

    w1t = np.ascontiguousarray(
        np.asarray(inputs["W1"], f32).transpose(2, 0, 1).reshape(4, H * D)
    ).astype(bf16)

    def mid(Wname):
        W = wT(inputs[Wname])  # [H, 256, 256]
        return np.ascontiguousarray(
            W.reshape(H, 2, 128, D).transpose(0, 2, 1, 3)
        ).astype(bf16)

    def bias(bname):
        b = np.asarray(inputs[bname], f32)  # [H, 256]
        return np.ascontiguousarray(b.reshape(H, 2, 128).transpose(2, 0, 1))

    w51T = wT(inputs["W51"])  # [H, 256, 2]
    w52T = wT(inputs["W52"])
    l5a = np.zeros((H, 2, 128, 4), f32)
    l5b = np.zeros((H, 2, 128, 4), f32)
    for kt in range(2):
        l5a[:, kt, :, 0:2] = -w51T[:, kt * 128 : (kt + 1) * 128, :]
        l5b[:, kt, :, 2:4] = w52T[:, kt * 128 : (kt + 1) * 128, :]
    l5a = l5a.astype(bf16)
    l5b = l5b.astype(bf16)

    shared = {
        "w1t": w1t,
        "w2": mid("W2"), "w31": mid("W31"), "w32": mid("W32"),
        "w41": mid("W41"), "w42": mid("W42"),
        "b1": bias("b1"), "b2": bias("b2"), "b31": bias("b31"),
        "b32": bias("b32"), "b41": bias("b41"), "b42": bias("b42"),
        "l5a": l5a, "l5b": l5b,
        "b51v": np.asarray(inputs["b51"], f32).reshape(1, 2 * H),
        "b52v": np.asarray(inputs["b52"], f32).reshape(1, 2 * H),
        "wt": np.asarray(inputs["wt"], f32).reshape(1, H),
        "mlab": np.asarray(inputs["mean_label"], f32).reshape(1, 2),
        "slab": np.asarray(inputs["std_label"], f32).reshape(1, 2),
        "meanp": np.asarray(inputs["mean"], f32).reshape(1, 4),
        "stdp": np.asarray(inputs["std"], f32).reshape(1, 4),
    }

    in_maps = []
    for c in range(NCORES):
        xs = x[c * BC : (c + 1) * BC]
        m = dict(shared)
        m["xT"] = np.ascontiguousarray(xs.T).astype(bf16)
        m["xg"] = np.ascontiguousarray(xs.reshape(NT, 128, 4).transpose(1, 0, 2))
        in_maps.append(m)
    return in_maps


_NC_CACHE = {}


def get_graph():
    if "nc" not in _NC_CACHE:
        _NC_CACHE["nc"] = build_graph()
    return _NC_CACHE["nc"]


def kernel(**inputs) -> np.ndarray:
    from concourse.bass_utils import run_bass_kernel_spmd

    nc = get_graph()
    in_maps = prep_inputs(inputs)
    res = run_bass_kernel_spmd(nc, in_maps, core_ids=list(range(NCORES)))
    return np.concatenate(
        [np.asarray(res.results[i]["out"], np.float32) for i in range(NCORES)], axis=0
    )


# revision 13
# speedup vs baseline: 1.0439x; 1.0439x over previous
"""Trainium2 Bass kernel for nn_ABNet: 10-head MLP + CBF-QP projection.

Data-parallel over 8 NeuronCores: batch 32768 -> 4096 rows/core, weights
replicated. MLP matmuls run in bf16 on the TensorEngine with activations in
[feature, batch] layout; the per-sample CBF geometry + QP halfspace
projection runs in fp32 on Vector/Scalar engines in batch-major layout.
"""

import sys

for _p in ("/opt/trn_rl_repo", "/root/.axon_site/_ro/trn_rl_repo"):
    if _p not in sys.path:
        sys.path.append(_p)

import numpy as np
import ml_dtypes

import concourse.bass as bass
import concourse.mybir as mybir
from concourse.tile import TileContext
from concourse.masks import make_identity

H = 10
B = 32768
NCORES = 8
BC = B // NCORES          # batch per core = 4096
NCHUNK = 8                # batch chunks per core
CH = BC // NCHUNK         # chunk width = 512
NT = BC // 128            # 128-wide batch tiles = 32
D = 256
L1C, L2C = 3.0, 3.0
OBS_X, OBS_Y, R = 0.0, 7.0, 4.0

F32 = mybir.dt.float32
BF16 = mybir.dt.bfloat16
AF = mybir.ActivationFunctionType
OP = mybir.AluOpType

MAX_WAITS = 1


def _split_waits(nc):
    """walrus in this env rejects >1 sync-wait per instruction; split extras
    onto preceding same-engine NOPs (semantically identical: sequential
    waits on an in-order engine)."""
    fn = nc.m.functions[0]
    for bb in fn.blocks:
        new = []
        for inst in bb.instructions:
            si = getattr(inst, "sync_info", None)
            if si is not None and si.on_wait and len(si.on_wait) > MAX_WAITS:
                waits = list(si.on_wait)
                while len(waits) > MAX_WAITS:
                    chunk, waits = waits[:MAX_WAITS], waits[MAX_WAITS:]
                    new.append(
                        mybir.InstNoOp(
                            name=nc.get_next_instruction_name(),
                            engine=inst.engine,
                            sync_info=mybir.SyncInfo(on_wait=chunk, on_update=[]),
                            bass_nofuse=True,
                        )
                    )
                si.on_wait = waits
            new.append(inst)
        bb.instructions[:] = new


def _bcast(t_ap, n=128):
    """Partition-broadcast AP for a DRAM tensor (step-0 partition dim)."""
    return bass.AP(tensor=t_ap.tensor, offset=t_ap.offset, ap=[[0, n]] + list(t_ap.ap))


def build_graph():
    nc = bass.Bass()
    dp = nc.declare_dram_parameter

    xT = dp("xT", [4, BC], BF16, isOutput=False)
    xg = dp("xg", [128, NT, 4], F32, isOutput=False)
    w1t = dp("w1t", [4, H * D], BF16, isOutput=False)
    # all 5 mid-layer weights for one head in a single DMA-able block
    whead = dp("whead", [H, 128, 5, 2, D], BF16, isOutput=False)
    bmid = {
        name: dp(name, [128, H, 2], F32, isOutput=False)
        for name in ("b1", "b2", "b31", "b32", "b41", "b42")
    }
    l5w = dp("l5w", [128, H, 2, 2, 4], BF16, isOutput=False)  # [p, h, branch, kt, col]
    b51v = dp("b51v", [1, 2 * H], F32, isOutput=False)
    b52v = dp("b52v", [1, 2 * H], F32, isOutput=False)
    wt = dp("wt", [1, H], F32, isOutput=False)
    mlab = dp("mlab", [1, 2], F32, isOutput=False)
    slab = dp("slab", [1, 2], F32, isOutput=False)
    meanp = dp("meanp", [1, 4], F32, isOutput=False)
    stdp = dp("stdp", [1, 4], F32, isOutput=False)
    out = dp("out", [BC, 2], F32, isOutput=True)

    wsm_dram = nc.dram_tensor("wsm_dram", [1, H], F32)

    with TileContext(nc) as tc:
        wpool = tc.alloc_tile_pool(name="weights", bufs=1)
        apool = tc.alloc_tile_pool(name="acts", bufs=1)
        qpool = tc.alloc_tile_pool(name="qp", bufs=1)
        ppool = tc.alloc_tile_pool(name="psum", bufs=4, space="PSUM")
        p5pool = tc.alloc_tile_pool(name="psum5", bufs=2, space="PSUM")
        ptpool = tc.alloc_tile_pool(name="psumT", bufs=2, space="PSUM")

        # ---- weight / input loads -------------------------------------
        # bulk loads go through gpsimd (SWDGE, 8 queues) so the SP HWDGE
        # FIFO isn't the serializer; small latency-critical loads first.
        xT_sb = wpool.tile([4, BC], BF16)
        nc.gpsimd.dma_start(out=xT_sb, in_=xT[:, :])
        xg_sb = qpool.tile([128, NT, 4], F32)
        nc.gpsimd.dma_start(out=xg_sb, in_=xg[:, :, :])

        bmid_sb = {}
        for name, t in bmid.items():
            btile = wpool.tile([128, H, 2], F32, name=f"{name}_sb")
            nc.gpsimd.dma_start(out=btile, in_=t[:, :, :])
            bmid_sb[name] = btile
        w1_sb = wpool.tile([4, H * D], BF16)
        nc.gpsimd.dma_start(out=w1_sb, in_=w1t[:, :])
        l5w_sb = wpool.tile([128, H, 2, 2, 4], BF16, name="l5w_sb")
        nc.gpsimd.dma_start(out=l5w_sb, in_=l5w[:, :, :, :, :])
        # one DMA per head: all 5 mid-layer weight blocks
        whead_sb = []
        for h in range(H):
            wtile = wpool.tile([128, 5, 2, D], BF16, name=f"whead_{h}")
            nc.gpsimd.dma_start(out=wtile, in_=whead[h])
            whead_sb.append(wtile)

        # broadcasts ([128, n] copies of small vectors)
        def bc_tile(src_ap, n, name):
            t = qpool.tile([128, n], F32, name=name)
            nc.gpsimd.dma_start(out=t, in_=_bcast(src_ap))
            return t

        b51_bc = bc_tile(b51v[0], 2 * H, "b51bc")
        b52_bc = bc_tile(b52v[0], 2 * H, "b52bc")
        mlab_bc = bc_tile(mlab[0], 2, "mlabbc")
        slab_bc = bc_tile(slab[0], 2, "slabbc")
        mean_bc = bc_tile(meanp[0], 4, "meanbc")
        std_bc = bc_tile(stdp[0], 4, "stdbc")

        ident = wpool.tile([128, 128], F32, name="ident")
        make_identity(nc, ident[:, :])

        # ---- softmax(wt) on device ------------------------------------
        wt_sb = qpool.tile([1, H], F32, name="wt_sb")
        nc.sync.dma_start(out=wt_sb, in_=wt[:, :])
        wexp = qpool.tile([1, H], F32, name="wexp")
        zeroh = qpool.tile([1, 1], F32, name="zeroh")
        nc.vector.memset(zeroh, 0.0)
        nc.scalar.activation(wexp, wt_sb, AF.Exp, bias=zeroh[:, 0:1])
        wsum = qpool.tile([1, 1], F32, name="wsum")
        nc.vector.reduce_sum(out=wsum, in_=wexp, axis=mybir.AxisListType.X)
        winv = qpool.tile([1, 1], F32, name="winv")
        nc.vector.reciprocal(winv, wsum)
        wnorm = qpool.tile([1, H], F32, name="wnorm")
        nc.vector.tensor_scalar(wnorm, wexp, winv[:, 0:1], None, OP.mult)
        nc.sync.dma_start(out=wsm_dram[:, :], in_=wnorm)
        w_bc = bc_tile(wsm_dram[0], H, "wbc")

        # ---- geometry (batch-major [128, NT] fp32) --------------------
        def qt(name):
            return qpool.tile([128, NT], F32, name=name)

        tt = nc.vector.tensor_tensor
        ts = nc.vector.tensor_scalar
        stt = nc.vector.scalar_tensor_tensor

        # de-normalized state rows
        t1q, w1q, t2q, w2q = qt("t1q"), qt("w1q"), qt("t2q"), qt("w2q")
        for dst, f in ((t1q, 0), (w1q, 1), (t2q, 2), (w2q, 3)):
            ts(dst, xg_sb[:, :, f], std_bc[:, f : f + 1], mean_bc[:, f : f + 1],
               OP.mult, OP.add)

        s1, c1, s2, c2 = qt("s1"), qt("c1"), qt("s2"), qt("c2")
        zero1 = qpool.tile([128, 1], F32, name="zero1")
        nc.vector.memset(zero1, 0.0)
        pi2 = qpool.tile([128, 1], F32, name="pi2")
        nc.vector.memset(pi2, float(np.pi / 2))
        nc.scalar.activation(s1, t1q, AF.Sin, bias=zero1[:, 0:1])
        nc.scalar.activation(c1, t1q, AF.Sin, bias=pi2[:, 0:1])
        nc.scalar.activation(s2, t2q, AF.Sin, bias=zero1[:, 0:1])
        nc.scalar.activation(c2, t2q, AF.Sin, bias=pi2[:, 0:1])

        px, py, vx, vy = qt("px"), qt("py"), qt("vx"), qt("vy")
        tmp1, tmp2, tmp3 = qt("tmp1"), qt("tmp2"), qt("tmp3")
        tt(tmp1, c1, c2, OP.add)
        ts(px, tmp1, L1C, -OBS_X, OP.mult, OP.add)
        tt(tmp1, s1, s2, OP.add)
        ts(py, tmp1, L1C, -OBS_Y, OP.mult, OP.add)
        tt(tmp1, s1, w1q, OP.mult)
        tt(tmp2, s2, w2q, OP.mult)
        tt(tmp3, tmp1, tmp2, OP.add)
        ts(vx, tmp3, -L1C, None, OP.mult)
        tt(tmp1, c1, w1q, OP.mult)
        tt(tmp2, c2, w2q, OP.mult)
        tt(tmp3, tmp1, tmp2, OP.add)
        ts(vy, tmp3, L1C, None, OP.mult)

        bar16, bdot4, lf2b = qt("bar16"), qt("bdot4"), qt("lf2b")
        tt(tmp1, px, px, OP.mult)
        tt(tmp2, py, py, OP.mult)
        tt(tmp3, tmp1, tmp2, OP.add)
        ts(bar16, tmp3, 16.0, -16.0 * R * R, OP.mult, OP.add)
        tt(tmp1, px, vx, OP.mult)
        tt(tmp2, py, vy, OP.mult)
        tt(tmp3, tmp1, tmp2, OP.add)
        ts(bdot4, tmp3, 8.0, None, OP.mult)

        w1sq, w2sq = qt("w1sq"), qt("w2sq")
        tt(w1sq, w1q, w1q, OP.mult)
        tt(w2sq, w2q, w2q, OP.mult)
        # lf2b = 2(vx^2 + vy^2) - 6*[px*(c1 w1sq + c2 w2sq) + py*(s1 w1sq + s2 w2sq)]
        tt(tmp1, c1, w1sq, OP.mult)
        tt(tmp2, c2, w2sq, OP.mult)
        tt(tmp3, tmp1, tmp2, OP.add)
        ua = qt("ua")
        tt(ua, px, tmp3, OP.mult)
        tt(tmp1, s1, w1sq, OP.mult)
        tt(tmp2, s2, w2sq, OP.mult)
        tt(tmp3, tmp1, tmp2, OP.add)
        ub = qt("ub")
        tt(ub, py, tmp3, OP.mult)
        tt(tmp1, ua, ub, OP.add)
        tt(tmp2, vx, vx, OP.mult)
        tt(tmp3, vy, vy, OP.mult)
        ud = qt("ud")
        tt(ud, tmp2, tmp3, OP.add)
        # lf2b = 2*ud - 6*tmp1  ->  (ud*2) add (tmp1*-6): two-step
        ts(tmp2, tmp1, -6.0, None, OP.mult)
        stt(lf2b, ud, 2.0, tmp2, OP.mult, OP.add)

        g1, g2, igg = qt("g1"), qt("g2"), qt("igg")
        tt(tmp1, px, s1, OP.mult)
        tt(tmp2, py, c1, OP.mult)
        tt(tmp3, tmp1, tmp2, OP.subtract)
        ts(g1, tmp3, 2.0 * L1C, None, OP.mult)
        tt(tmp1, px, s2, OP.mult)
        tt(tmp2, py, c2, OP.mult)
        tt(tmp3, tmp1, tmp2, OP.subtract)
        ts(g2, tmp3, 2.0 * L2C, None, OP.mult)
        tt(tmp1, g1, g1, OP.mult)
        tt(tmp2, g2, g2, OP.mult)
        tt(tmp3, tmp1, tmp2, OP.add)
        nc.vector.reciprocal(igg, tmp3)

        # ---- MLP head loop --------------------------------------------
        x5all = qpool.tile([4 * H, BC], F32, name="x5all")

        def evac(engine_idx, dst_ap, psum_ap, bias_ap):
            """relu(psum + bias) -> bf16 dst; engine picked by parity."""
            if engine_idx % 2 == 0:
                nc.scalar.activation(dst_ap, psum_ap, AF.Relu, bias=bias_ap)
            else:
                nc.vector.tensor_scalar(dst_ap, psum_ap, bias_ap, 0.0, OP.add, OP.max)

        for h in range(H):
            a1 = apool.tile([128, 2, BC], BF16, tag="t_x1", name=f"x1_{h}")
            a2 = apool.tile([128, 2, BC], BF16, tag="t_x2", name=f"x2_{h}")
            a31 = apool.tile([128, 2, BC], BF16, tag="t_x31", name=f"x31_{h}")
            a32 = apool.tile([128, 2, BC], BF16, tag="t_x32", name=f"x32_{h}")
            a41 = apool.tile([128, 2, BC], BF16, tag="t_x41", name=f"x41_{h}")
            a42 = apool.tile([128, 2, BC], BF16, tag="t_x42", name=f"x42_{h}")

            # L1: K=4
            for mt in range(2):
                for c in range(NCHUNK):
                    ps = ppool.tile([128, CH], F32)
                    nc.tensor.matmul(
                        ps,
                        w1_sb[:, h * D + mt * 128 : h * D + (mt + 1) * 128],
                        xT_sb[:, c * CH : (c + 1) * CH],
                        start=True, stop=True,
                    )
                    evac(mt, a1[:, mt, c * CH : (c + 1) * CH], ps,
                         bmid_sb["b1"][:, h, mt : mt + 1])

            # mid layers
            for li, (bname, src, dst) in enumerate((
                ("b2", a1, a2),
                ("b31", a2, a31),
                ("b32", a2, a32),
                ("b41", a31, a41),
                ("b42", a32, a42),
            )):
                wtiles = whead_sb[h]
                btile = bmid_sb[bname]
                for mt in range(2):
                    for c in range(NCHUNK):
                        ps = ppool.tile([128, CH], F32)
                        for kt in range(2):
                            nc.tensor.matmul(
                                ps,
                                wtiles[:, li, kt, mt * 128 : (mt + 1) * 128],
                                src[:, kt, c * CH : (c + 1) * CH],
                                start=(kt == 0), stop=(kt == 1),
                            )
                        evac(mt, dst[:, mt, c * CH : (c + 1) * CH], ps,
                             btile[:, h, mt : mt + 1])

            # L5: [-W51^T x41 | W52^T x42] -> psum [4, CH]
            for c in range(NCHUNK):
                ps5 = p5pool.tile([4, CH], F32)
                nc.tensor.matmul(ps5, l5w_sb[:, h, 0, 0, :], a41[:, 0, c * CH : (c + 1) * CH],
                                 start=True, stop=False)
                nc.tensor.matmul(ps5, l5w_sb[:, h, 0, 1, :], a41[:, 1, c * CH : (c + 1) * CH],
                                 start=False, stop=False)
                nc.tensor.matmul(ps5, l5w_sb[:, h, 1, 0, :], a42[:, 0, c * CH : (c + 1) * CH],
                                 start=False, stop=False)
                nc.tensor.matmul(ps5, l5w_sb[:, h, 1, 1, :], a42[:, 1, c * CH : (c + 1) * CH],
                                 start=False, stop=True)
                stg5 = apool.tile([4, CH], F32, tag="t_stg5", bufs=4)
                if c % 2 == 0:
                    nc.scalar.activation(stg5, ps5, AF.Copy)
                else:
                    nc.vector.tensor_copy(stg5, ps5)
                nc.gpsimd.dma_start(
                    out=x5all[4 * h : 4 * h + 4, c * CH : (c + 1) * CH], in_=stg5
                )

        # ---- transpose x5all [40, BC] -> x5T [128, NT, 40] ------------
        x5T = qpool.tile([128, NT, 4 * H], F32, name="x5T")
        for t in range(NT):
            pst = ptpool.tile([128, 4 * H], F32)
            nc.tensor.transpose(
                pst, x5all[:, t * 128 : (t + 1) * 128], ident[0 : 4 * H, 0 : 4 * H]
            )
            nc.vector.tensor_copy(x5T[:, t, :], pst)

        # ---- QP projection --------------------------------------------
        # x5T cols per head h: 0:-(W51 x41) c0, 1: c1, 2: z52 c0, 3: z52 c1
        # u_unc = col - b51 ; sigma = sigmoid(z52 + b52)
        for h in range(H):
            for cix in range(2):
                ts(x5T[:, :, 4 * h + cix], x5T[:, :, 4 * h + cix],
                   b51_bc[:, 2 * h + cix : 2 * h + cix + 1], None, OP.subtract)
                nc.scalar.activation(
                    x5T[:, :, 4 * h + 2 + cix], x5T[:, :, 4 * h + 2 + cix],
                    AF.Sigmoid, bias=b52_bc[:, 2 * h + cix : 2 * h + cix + 1],
                )

        # shared per-sample terms using head-0's sigma_a
        s0 = x5T[:, :, 2]
        Aq, Dq = qt("Aq"), qt("Dq")
        tt(tmp1, s0, bdot4, OP.mult)
        tt(Aq, lf2b, tmp1, OP.add)
        tt(tmp1, s0, bar16, OP.mult)
        tt(Dq, bdot4, tmp1, OP.add)

        acc_e, acc_u1, acc_u2 = qt("acc_e"), qt("acc_u1"), qt("acc_u2")
        nc.vector.memset(acc_e, 0.0)
        nc.vector.memset(acc_u1, 0.0)
        nc.vector.memset(acc_u2, 0.0)

        hq, e_q = qt("hq"), qt("e_q")
        for h in range(H):
            u1 = x5T[:, :, 4 * h + 0]
            u2 = x5T[:, :, 4 * h + 1]
            sb_i = x5T[:, :, 4 * h + 3]
            # h_i = A + sb_i * D
            tt(tmp1, sb_i, Dq, OP.mult)
            tt(hq, Aq, tmp1, OP.add)
            # viol = u1*g1 + u2*g2 - h_i ; e = max(viol, 0)
            tt(tmp1, u1, g1, OP.mult)
            tt(tmp2, u2, g2, OP.mult)
            tt(tmp3, tmp1, tmp2, OP.add)
            stt(tmp1, hq, -1.0, tmp3, OP.mult, OP.add)
            ts(e_q, tmp1, 0.0, None, OP.max)
            # weighted accumulation
            wh = w_bc[:, h : h + 1]
            stt(acc_e, e_q, wh, acc_e, OP.mult, OP.add)
            stt(acc_u1, u1, wh, acc_u1, OP.mult, OP.add)
            stt(acc_u2, u2, wh, acc_u2, OP.mult, OP.add)

        # out_c = (acc_uc - acc_e*igg*g_c - mlab_c) / slab_c
        lamw = qt("lamw")
        tt(lamw, acc_e, igg, OP.mult)
        isl = qpool.tile([128, 2], F32, name="isl")
        nc.vector.reciprocal(isl, slab_bc)
        out_t = qpool.tile([128, NT, 2], F32, name="out_t")
        for cix, (accu, g_c) in enumerate(((acc_u1, g1), (acc_u2, g2))):
            tt(tmp1, lamw, g_c, OP.mult)
            tt(tmp2, accu, tmp1, OP.subtract)
            ts(out_t[:, :, cix], tmp2, mlab_bc[:, cix : cix + 1],
               isl[:, cix : cix + 1], OP.subtract, OP.mult)

        nc.sync.dma_start(
            out=out.rearrange("(t p) c -> p t c", p=128), in_=out_t
        )

        for pool in (ptpool, p5pool, ppool, qpool, apool, wpool):
            pool.release()

    _split_waits(nc)
    return nc


def prep_inputs(inputs):
    """Host-side shard + layout prep. Returns in_maps for 8 cores."""
    f32 = np.float32
    bf16 = ml_dtypes.bfloat16
    x = np.asarray(inputs["x"], f32)

    def wT(W):  # [H, dout, din] -> [H, din, dout]
        return np.ascontiguousarray(np.asarray(W, f32).transpose(0, 2, 1))

    w1t = np.ascontiguousarray(
        np.asarray(inputs["W1"], f32).transpose(2, 0, 1).reshape(4, H * D)
    ).astype(bf16)

    def mid(Wname):
        W = wT(inputs[Wname])  # [H, 256, 256]
        return W.reshape(H, 2, 128, D).transpose(0, 2, 1, 3)  # [H, 128, 2, D]

    # [H, 128, 5(layer), 2(kt), D]
    whead = np.ascontiguousarray(
        np.stack([mid(n) for n in ("W2", "W31", "W32", "W41", "W42")], axis=2)
    ).astype(bf16)

    def bias(bname):
        b = np.asarray(inputs[bname], f32)  # [H, 256]
        return np.ascontiguousarray(b.reshape(H, 2, 128).transpose(2, 0, 1))

    w51T = wT(inputs["W51"])  # [H, 256, 2]
    w52T = wT(inputs["W52"])
    # [128, H, branch, kt, 4]
    l5wv = np.zeros((128, H, 2, 2, 4), f32)
    for kt in range(2):
        ksl = slice(kt * 128, (kt + 1) * 128)
        l5wv[:, :, 0, kt, 0:2] = -w51T[:, ksl, :].transpose(1, 0, 2)
        l5wv[:, :, 1, kt, 2:4] = w52T[:, ksl, :].transpose(1, 0, 2)
    l5wv = np.ascontiguousarray(l5wv).astype(bf16)

    shared = {
        "w1t": w1t,
        "whead": whead,
        "b1": bias("b1"), "b2": bias("b2"), "b31": bias("b31"),
        "b32": bias("b32"), "b41": bias("b41"), "b42": bias("b42"),
        "l5w": l5wv,
        "b51v": np.asarray(inputs["b51"], f32).reshape(1, 2 * H),
        "b52v": np.asarray(inputs["b52"], f32).reshape(1, 2 * H),
        "wt": np.asarray(inputs["wt"], f32).reshape(1, H),
        "mlab": np.asarray(inputs["mean_label"], f32).reshape(1, 2),
        "slab": np.asarray(inputs["std_label"], f32).reshape(1, 2),
        "meanp": np.asarray(inputs["mean"], f32).reshape(1, 4),
        "stdp": np.asarray(inputs["std"], f32).reshape(1, 4),
    }

    in_maps = []
    for c in range(NCORES):
        xs = x[c * BC : (c + 1) * BC]
        m = dict(shared)
        m["xT"] = np.ascontiguousarray(xs.T).astype(bf16)
        m["xg"] = np.ascontiguousarray(xs.reshape(NT, 128, 4).transpose(1, 0, 2))
        in_maps.append(m)
    return in_maps


_NC_CACHE = {}


def get_graph():
    if "nc" not in _NC_CACHE:
        _NC_CACHE["nc"] = build_graph()
    return _NC_CACHE["nc"]


def kernel(**inputs) -> np.ndarray:
    from concourse.bass_utils import run_bass_kernel_spmd

    nc = get_graph()
    in_maps = prep_inputs(inputs)
    res = run_bass_kernel_spmd(nc, in_maps, core_ids=list(range(NCORES)))
    return np.concatenate(
        [np.asarray(res.results[i]["out"], np.float32) for i in range(NCORES)], axis=0
    )


# revision 14
# speedup vs baseline: 1.4128x; 1.3534x over previous
"""Trainium2 Bass kernel for nn_ABNet: 10-head MLP + CBF-QP projection.

Data-parallel over 8 NeuronCores: batch 32768 -> 4096 rows/core, weights
replicated. MLP matmuls run in bf16 on the TensorEngine with activations in
[feature, batch] layout; the per-sample CBF geometry + QP halfspace
projection runs in fp32 on Vector/Scalar engines in batch-major layout.
The batch is processed in two halves so the second half's matmuls overlap
the first half's transpose/QP epilogue.
"""

import sys

for _p in ("/opt/trn_rl_repo", "/root/.axon_site/_ro/trn_rl_repo"):
    if _p not in sys.path:
        sys.path.append(_p)

import numpy as np
import ml_dtypes

import concourse.bass as bass
import concourse.mybir as mybir
from concourse.tile import TileContext
from concourse.masks import make_identity

H = 10
B = 32768
NCORES = 8
BC = B // NCORES          # batch per core = 4096
NHALF = 2                 # batch halves per core
BH = BC // NHALF          # 2048
CH = 512                  # matmul moving chunk (one PSUM bank)
NCH = BH // CH            # chunks per half = 4
NT = BC // 128            # 128-wide batch tiles = 32
NTH = BH // 128           # per half = 16
D = 256
L1C, L2C = 3.0, 3.0
OBS_X, OBS_Y, R = 0.0, 7.0, 4.0

F32 = mybir.dt.float32
BF16 = mybir.dt.bfloat16
AF = mybir.ActivationFunctionType
OP = mybir.AluOpType

MAX_WAITS = 1


def _split_waits(nc):
    """walrus in this env rejects >1 sync-wait per instruction; split extras
    onto preceding same-engine NOPs (semantically identical: sequential
    waits on an in-order engine)."""
    fn = nc.m.functions[0]
    for bb in fn.blocks:
        new = []
        for inst in bb.instructions:
            si = getattr(inst, "sync_info", None)
            if si is not None and si.on_wait and len(si.on_wait) > MAX_WAITS:
                waits = list(si.on_wait)
                while len(waits) > MAX_WAITS:
                    chunk, waits = waits[:MAX_WAITS], waits[MAX_WAITS:]
                    new.append(
                        mybir.InstNoOp(
                            name=nc.get_next_instruction_name(),
                            engine=inst.engine,
                            sync_info=mybir.SyncInfo(on_wait=chunk, on_update=[]),
                            bass_nofuse=True,
                        )
                    )
                si.on_wait = waits
            new.append(inst)
        bb.instructions[:] = new


def _bcast(t_ap, n=128):
    """Partition-broadcast AP for a DRAM tensor (step-0 partition dim)."""
    return bass.AP(tensor=t_ap.tensor, offset=t_ap.offset, ap=[[0, n]] + list(t_ap.ap))


def build_graph():
    nc = bass.Bass()
    dp = nc.declare_dram_parameter

    xT = dp("xT", [4, BC], BF16, isOutput=False)
    xg = dp("xg", [128, NT, 4], F32, isOutput=False)
    w1t = dp("w1t", [4, H * D], BF16, isOutput=False)
    # all 5 mid-layer weights for one head in a single DMA-able block
    whead = dp("whead", [H, 128, 5, 2, D], BF16, isOutput=False)
    bmid = {
        name: dp(name, [128, H, 2], F32, isOutput=False)
        for name in ("b1", "b2", "b31", "b32", "b41", "b42")
    }
    l5w = dp("l5w", [128, H, 2, 2, 4], BF16, isOutput=False)  # [p,h,branch,kt,col]
    b51v = dp("b51v", [1, 2 * H], F32, isOutput=False)
    b52v = dp("b52v", [1, 2 * H], F32, isOutput=False)
    wt = dp("wt", [1, H], F32, isOutput=False)
    mlab = dp("mlab", [1, 2], F32, isOutput=False)
    slab = dp("slab", [1, 2], F32, isOutput=False)
    meanp = dp("meanp", [1, 4], F32, isOutput=False)
    stdp = dp("stdp", [1, 4], F32, isOutput=False)
    out = dp("out", [BC, 2], F32, isOutput=True)

    wsm_dram = nc.dram_tensor("wsm_dram", [1, H], F32)

    with TileContext(nc) as tc:
        wpool = tc.alloc_tile_pool(name="weights", bufs=1)
        apool = tc.alloc_tile_pool(name="acts", bufs=1)
        qpool = tc.alloc_tile_pool(name="qp", bufs=1)
        ppool = tc.alloc_tile_pool(name="psum", bufs=6, space="PSUM")
        spool = tc.alloc_tile_pool(name="psmall", bufs=2, space="PSUM")

        # ---- loads ------------------------------------------------------
        # bulk loads via gpsimd (SWDGE, 8 queues) so the SP HWDGE FIFO
        # isn't a serializer; consumption-order emission.
        # L1 operands are replicated at partition bases 0/32/64/96 for
        # row-group-packed (4x concurrent) K=4 matmuls.
        xT4_sb = wpool.tile([128, BC], BF16, name="xT4")
        w14_sb = wpool.tile([128, H * D], BF16, name="w14")
        for g in range(4):
            nc.gpsimd.dma_start(out=xT4_sb[32 * g : 32 * g + 4, :], in_=xT[:, :])
            nc.gpsimd.dma_start(out=w14_sb[32 * g : 32 * g + 4, :], in_=w1t[:, :])
        bmid_sb = {}
        for name, t in bmid.items():
            btile = wpool.tile([128, H, 2], F32, name=f"{name}_sb")
            nc.gpsimd.dma_start(out=btile, in_=t[:, :, :])
            bmid_sb[name] = btile
        l5w_sb = wpool.tile([128, H, 2, 2, 4], BF16, name="l5w_sb")
        nc.gpsimd.dma_start(out=l5w_sb, in_=l5w[:, :, :, :, :])
        xg_sb = qpool.tile([128, NT, 4], F32)
        nc.gpsimd.dma_start(out=xg_sb, in_=xg[:, :, :])
        whead_sb = []
        for h in range(H):
            wtile = wpool.tile([128, 5, 2, D], BF16, name=f"whead_{h}")
            nc.gpsimd.dma_start(out=wtile, in_=whead[h])
            whead_sb.append(wtile)

        def bc_tile(src_ap, n, name):
            t = qpool.tile([128, n], F32, name=name)
            nc.gpsimd.dma_start(out=t, in_=_bcast(src_ap))
            return t

        b51_bc = bc_tile(b51v[0], 2 * H, "b51bc")
        b52_bc = bc_tile(b52v[0], 2 * H, "b52bc")
        mlab_bc = bc_tile(mlab[0], 2, "mlabbc")
        slab_bc = bc_tile(slab[0], 2, "slabbc")
        mean_bc = bc_tile(meanp[0], 4, "meanbc")
        std_bc = bc_tile(stdp[0], 4, "stdbc")

        ident = wpool.tile([128, 128], F32, name="ident")
        make_identity(nc, ident[:, :])

        # ---- softmax(wt) -------------------------------------------------
        wt_sb = qpool.tile([1, H], F32, name="wt_sb")
        nc.sync.dma_start(out=wt_sb, in_=wt[:, :])
        wexp = qpool.tile([1, H], F32, name="wexp")
        zeroh = qpool.tile([1, 1], F32, name="zeroh")
        nc.vector.memset(zeroh, 0.0)
        nc.scalar.activation(wexp, wt_sb, AF.Exp, bias=zeroh[:, 0:1])
        wsum = qpool.tile([1, 1], F32, name="wsum")
        nc.vector.reduce_sum(out=wsum, in_=wexp, axis=mybir.AxisListType.X)
        winv = qpool.tile([1, 1], F32, name="winv")
        nc.vector.reciprocal(winv, wsum)
        wnorm = qpool.tile([1, H], F32, name="wnorm")
        nc.vector.tensor_scalar(wnorm, wexp, winv[:, 0:1], None, OP.mult)
        nc.sync.dma_start(out=wsm_dram[:, :], in_=wnorm)
        w_bc = bc_tile(wsm_dram[0], H, "wbc")

        # ---- geometry (batch-major [128, NT] fp32, full width) -----------
        def qt(name):
            return qpool.tile([128, NT], F32, name=name)

        tt = nc.vector.tensor_tensor
        ts = nc.vector.tensor_scalar
        stt = nc.vector.scalar_tensor_tensor

        t1q, w1q, t2q, w2q = qt("t1q"), qt("w1q"), qt("t2q"), qt("w2q")
        for dst, f in ((t1q, 0), (w1q, 1), (t2q, 2), (w2q, 3)):
            ts(dst, xg_sb[:, :, f], std_bc[:, f : f + 1], mean_bc[:, f : f + 1],
               OP.mult, OP.add)

        s1, c1, s2, c2 = qt("s1"), qt("c1"), qt("s2"), qt("c2")
        zero1 = qpool.tile([128, 1], F32, name="zero1")
        nc.vector.memset(zero1, 0.0)
        pi2 = qpool.tile([128, 1], F32, name="pi2")
        nc.vector.memset(pi2, float(np.pi / 2))
        nc.scalar.activation(s1, t1q, AF.Sin, bias=zero1[:, 0:1])
        nc.scalar.activation(c1, t1q, AF.Sin, bias=pi2[:, 0:1])
        nc.scalar.activation(s2, t2q, AF.Sin, bias=zero1[:, 0:1])
        nc.scalar.activation(c2, t2q, AF.Sin, bias=pi2[:, 0:1])

        px, py, vx, vy = qt("px"), qt("py"), qt("vx"), qt("vy")
        tmp1, tmp2, tmp3 = qt("tmp1"), qt("tmp2"), qt("tmp3")
        tt(tmp1, c1, c2, OP.add)
        ts(px, tmp1, L1C, -OBS_X, OP.mult, OP.add)
        tt(tmp1, s1, s2, OP.add)
        ts(py, tmp1, L1C, -OBS_Y, OP.mult, OP.add)
        tt(tmp1, s1, w1q, OP.mult)
        tt(tmp2, s2, w2q, OP.mult)
        tt(tmp3, tmp1, tmp2, OP.add)
        ts(vx, tmp3, -L1C, None, OP.mult)
        tt(tmp1, c1, w1q, OP.mult)
        tt(tmp2, c2, w2q, OP.mult)
        tt(tmp3, tmp1, tmp2, OP.add)
        ts(vy, tmp3, L1C, None, OP.mult)

        bar16, bdot4, lf2b = qt("bar16"), qt("bdot4"), qt("lf2b")
        tt(tmp1, px, px, OP.mult)
        tt(tmp2, py, py, OP.mult)
        tt(tmp3, tmp1, tmp2, OP.add)
        ts(bar16, tmp3, 16.0, -16.0 * R * R, OP.mult, OP.add)
        tt(tmp1, px, vx, OP.mult)
        tt(tmp2, py, vy, OP.mult)
        tt(tmp3, tmp1, tmp2, OP.add)
        ts(bdot4, tmp3, 8.0, None, OP.mult)

        w1sq, w2sq = qt("w1sq"), qt("w2sq")
        tt(w1sq, w1q, w1q, OP.mult)
        tt(w2sq, w2q, w2q, OP.mult)
        tt(tmp1, c1, w1sq, OP.mult)
        tt(tmp2, c2, w2sq, OP.mult)
        tt(tmp3, tmp1, tmp2, OP.add)
        ua = qt("ua")
        tt(ua, px, tmp3, OP.mult)
        tt(tmp1, s1, w1sq, OP.mult)
        tt(tmp2, s2, w2sq, OP.mult)
        tt(tmp3, tmp1, tmp2, OP.add)
        ub = qt("ub")
        tt(ub, py, tmp3, OP.mult)
        tt(tmp1, ua, ub, OP.add)
        tt(tmp2, vx, vx, OP.mult)
        tt(tmp3, vy, vy, OP.mult)
        ud = qt("ud")
        tt(ud, tmp2, tmp3, OP.add)
        ts(tmp2, tmp1, -6.0, None, OP.mult)
        stt(lf2b, ud, 2.0, tmp2, OP.mult, OP.add)

        g1, g2, igg = qt("g1"), qt("g2"), qt("igg")
        tt(tmp1, px, s1, OP.mult)
        tt(tmp2, py, c1, OP.mult)
        tt(tmp3, tmp1, tmp2, OP.subtract)
        ts(g1, tmp3, 2.0 * L1C, None, OP.mult)
        tt(tmp1, px, s2, OP.mult)
        tt(tmp2, py, c2, OP.mult)
        tt(tmp3, tmp1, tmp2, OP.subtract)
        ts(g2, tmp3, 2.0 * L2C, None, OP.mult)
        tt(tmp1, g1, g1, OP.mult)
        tt(tmp2, g2, g2, OP.mult)
        tt(tmp3, tmp1, tmp2, OP.add)
        nc.vector.reciprocal(igg, tmp3)

        isl = qpool.tile([128, 2], F32, name="isl")
        nc.vector.reciprocal(isl, slab_bc)

        def evac(engine_idx, dst_ap, psum_ap, bias_ap):
            """relu(psum + bias) -> bf16 dst; engine picked by parity."""
            if engine_idx % 2 == 0:
                nc.scalar.activation(dst_ap, psum_ap, AF.Relu, bias=bias_ap)
            else:
                nc.vector.tensor_scalar(dst_ap, psum_ap, bias_ap, 0.0, OP.add, OP.max)

        out_r = out.rearrange("(t p) c -> p t c", p=128)

        # ================= per-half pipeline ==============================
        for half in range(NHALF):
            hb = half * BH  # batch offset of this half

            x5all = qpool.tile([4 * H, BH], F32, name=f"x5all_{half}")

            for h in range(H):
                a1 = apool.tile([128, 2, BH], BF16, tag="t_x1", name=f"x1_{half}_{h}")
                a2 = apool.tile([128, 2, BH], BF16, tag="t_x2", name=f"x2_{half}_{h}")
                a31 = apool.tile([128, 2, BH], BF16, tag="t_x31", name=f"x31_{half}_{h}")
                a32 = apool.tile([128, 2, BH], BF16, tag="t_x32", name=f"x32_{half}_{h}")
                a41 = apool.tile([128, 2, BH], BF16, tag="t_x41", name=f"x41_{half}_{h}")
                a42 = apool.tile([128, 2, BH], BF16, tag="t_x42", name=f"x42_{half}_{h}")

                # L1 (K=4): row-group packed, 4 concurrent matmuls
                pairs = [(mt, c) for mt in range(2) for c in range(NCH)]
                for gbase in range(0, len(pairs), 4):
                    group = pairs[gbase : gbase + 4]
                    pss = []
                    for j, (mt, c) in enumerate(group):
                        ps = ppool.tile([128, CH], F32)
                        nc.tensor.matmul(
                            ps,
                            w14_sb[32 * j : 32 * j + 4,
                                   h * D + mt * 128 : h * D + (mt + 1) * 128],
                            xT4_sb[32 * j : 32 * j + 4,
                                   hb + c * CH : hb + (c + 1) * CH],
                            start=True, stop=True,
                            tile_position=(32 * j, 0),
                        )
                        pss.append(ps)
                    for j, (mt, c) in enumerate(group):
                        evac(mt, a1[:, mt, c * CH : (c + 1) * CH], pss[j],
                             bmid_sb["b1"][:, h, mt : mt + 1])

                # mid layers
                for li, (bname, src, dst) in enumerate((
                    ("b2", a1, a2),
                    ("b31", a2, a31),
                    ("b32", a2, a32),
                    ("b41", a31, a41),
                    ("b42", a32, a42),
                )):
                    wtiles = whead_sb[h]
                    btile = bmid_sb[bname]
                    for mt in range(2):
                        for c in range(NCH):
                            ps = ppool.tile([128, CH], F32)
                            for kt in range(2):
                                nc.tensor.matmul(
                                    ps,
                                    wtiles[:, li, kt, mt * 128 : (mt + 1) * 128],
                                    src[:, kt, c * CH : (c + 1) * CH],
                                    start=(kt == 0), stop=(kt == 1),
                                )
                            evac(mt, dst[:, mt, c * CH : (c + 1) * CH], ps,
                                 btile[:, h, mt : mt + 1])

                # L5: [-W51^T x41 | W52^T x42] -> psum [4, CH]
                for c in range(NCH):
                    ps5 = spool.tile([4, CH], F32, tag="small")
                    nc.tensor.matmul(ps5, l5w_sb[:, h, 0, 0, :],
                                     a41[:, 0, c * CH : (c + 1) * CH],
                                     start=True, stop=False)
                    nc.tensor.matmul(ps5, l5w_sb[:, h, 0, 1, :],
                                     a41[:, 1, c * CH : (c + 1) * CH],
                                     start=False, stop=False)
                    nc.tensor.matmul(ps5, l5w_sb[:, h, 1, 0, :],
                                     a42[:, 0, c * CH : (c + 1) * CH],
                                     start=False, stop=False)
                    nc.tensor.matmul(ps5, l5w_sb[:, h, 1, 1, :],
                                     a42[:, 1, c * CH : (c + 1) * CH],
                                     start=False, stop=True)
                    stg5 = apool.tile([4, CH], F32, tag="t_stg5", bufs=4)
                    if c % 2 == 0:
                        nc.scalar.activation(stg5, ps5, AF.Copy)
                    else:
                        nc.vector.tensor_copy(stg5, ps5)
                    nc.gpsimd.dma_start(
                        out=x5all[4 * h : 4 * h + 4, c * CH : (c + 1) * CH],
                        in_=stg5,
                    )

            # ---- transpose x5all [40, BH] -> x5T [128, NTH, 40] ----------
            x5T = qpool.tile([128, NTH, 4 * H], F32, name=f"x5T_{half}")
            for t in range(NTH):
                pst = spool.tile([128, 4 * H], F32, tag="small")
                nc.tensor.transpose(
                    pst, x5all[:, t * 128 : (t + 1) * 128],
                    ident[0 : 4 * H, 0 : 4 * H],
                )
                nc.vector.tensor_copy(x5T[:, t, :], pst)

            # ---- QP projection ------------------------------------------
            hsl = slice(half * NTH, (half + 1) * NTH)  # geometry slice

            for h in range(H):
                for cix in range(2):
                    ts(x5T[:, :, 4 * h + cix], x5T[:, :, 4 * h + cix],
                       b51_bc[:, 2 * h + cix : 2 * h + cix + 1], None, OP.subtract)
                    nc.scalar.activation(
                        x5T[:, :, 4 * h + 2 + cix], x5T[:, :, 4 * h + 2 + cix],
                        AF.Sigmoid, bias=b52_bc[:, 2 * h + cix : 2 * h + cix + 1],
                    )

            def qth(name):
                return qpool.tile([128, NTH], F32, name=f"{name}_{half}")

            s0 = x5T[:, :, 2]
            Aq, Dq = qth("Aq"), qth("Dq")
            htmp1, htmp2, htmp3 = qth("htmp1"), qth("htmp2"), qth("htmp3")
            tt(htmp1, s0, bdot4[:, hsl], OP.mult)
            tt(Aq, lf2b[:, hsl], htmp1, OP.add)
            tt(htmp1, s0, bar16[:, hsl], OP.mult)
            tt(Dq, bdot4[:, hsl], htmp1, OP.add)

            acc_e, acc_u1, acc_u2 = qth("acc_e"), qth("acc_u1"), qth("acc_u2")
            nc.vector.memset(acc_e, 0.0)
            nc.vector.memset(acc_u1, 0.0)
            nc.vector.memset(acc_u2, 0.0)

            hq, e_q = qth("hq"), qth("e_q")
            for h in range(H):
                u1 = x5T[:, :, 4 * h + 0]
                u2 = x5T[:, :, 4 * h + 1]
                sb_i = x5T[:, :, 4 * h + 3]
                tt(htmp1, sb_i, Dq, OP.mult)
                tt(hq, Aq, htmp1, OP.add)
                tt(htmp1, u1, g1[:, hsl], OP.mult)
                tt(htmp2, u2, g2[:, hsl], OP.mult)
                tt(htmp3, htmp1, htmp2, OP.add)
                stt(htmp1, hq, -1.0, htmp3, OP.mult, OP.add)
                ts(e_q, htmp1, 0.0, None, OP.max)
                wh = w_bc[:, h : h + 1]
                stt(acc_e, e_q, wh, acc_e, OP.mult, OP.add)
                stt(acc_u1, u1, wh, acc_u1, OP.mult, OP.add)
                stt(acc_u2, u2, wh, acc_u2, OP.mult, OP.add)

            lamw = qth("lamw")
            tt(lamw, acc_e, igg[:, hsl], OP.mult)
            out_t = qpool.tile([128, NTH, 2], F32, name=f"out_t_{half}")
            for cix, (accu, g_c) in enumerate(((acc_u1, g1), (acc_u2, g2))):
                tt(htmp1, lamw, g_c[:, hsl], OP.mult)
                tt(htmp2, accu, htmp1, OP.subtract)
                ts(out_t[:, :, cix], htmp2, mlab_bc[:, cix : cix + 1],
                   isl[:, cix : cix + 1], OP.subtract, OP.mult)

            nc.sync.dma_start(out=out_r[:, hsl, :], in_=out_t)

        for pool in (spool, ppool, qpool, apool, wpool):
            pool.release()

    _split_waits(nc)
    return nc


def prep_inputs(inputs):
    """Host-side shard + layout prep. Returns in_maps for 8 cores."""
    f32 = np.float32
    bf16 = ml_dtypes.bfloat16
    x = np.asarray(inputs["x"], f32)

    def wT(W):  # [H, dout, din] -> [H, din, dout]
        return np.ascontiguousarray(np.asarray(W, f32).transpose(0, 2, 1))

    w1t = np.ascontiguousarray(
        np.asarray(inputs["W1"], f32).transpose(2, 0, 1).reshape(4, H * D)
    ).astype(bf16)

    def mid(Wname):
        W = wT(inputs[Wname])  # [H, 256, 256]
        return W.reshape(H, 2, 128, D).transpose(0, 2, 1, 3)  # [H, 128, 2, D]

    # [H, 128, 5(layer), 2(kt), D]
    whead = np.ascontiguousarray(
        np.stack([mid(n) for n in ("W2", "W31", "W32", "W41", "W42")], axis=2)
    ).astype(bf16)

    def bias(bname):
        b = np.asarray(inputs[bname], f32)  # [H, 256]
        return np.ascontiguousarray(b.reshape(H, 2, 128).transpose(2, 0, 1))

    w51T = wT(inputs["W51"])  # [H, 256, 2]
    w52T = wT(inputs["W52"])
    # [128, H, branch, kt, 4]
    l5wv = np.zeros((128, H, 2, 2, 4), f32)
    for kt in range(2):
        ksl = slice(kt * 128, (kt + 1) * 128)
        l5wv[:, :, 0, kt, 0:2] = -w51T[:, ksl, :].transpose(1, 0, 2)
        l5wv[:, :, 1, kt, 2:4] = w52T[:, ksl, :].transpose(1, 0, 2)
    l5wv = np.ascontiguousarray(l5wv).astype(bf16)

    shared = {
        "w1t": w1t,
        "whead": whead,
        "b1": bias("b1"), "b2": bias("b2"), "b31": bias("b31"),
        "b32": bias("b32"), "b41": bias("b41"), "b42": bias("b42"),
        "l5w": l5wv,
        "b51v": np.asarray(inputs["b51"], f32).reshape(1, 2 * H),
        "b52v": np.asarray(inputs["b52"], f32).reshape(1, 2 * H),
        "wt": np.asarray(inputs["wt"], f32).reshape(1, H),
        "mlab": np.asarray(inputs["mean_label"], f32).reshape(1, 2),
        "slab": np.asarray(inputs["std_label"], f32).reshape(1, 2),
        "meanp": np.asarray(inputs["mean"], f32).reshape(1, 4),
        "stdp": np.asarray(inputs["std"], f32).reshape(1, 4),
    }

    in_maps = []
    for c in range(NCORES):
        xs = x[c * BC : (c + 1) * BC]
        m = dict(shared)
        m["xT"] = np.ascontiguousarray(xs.T).astype(bf16)
        m["xg"] = np.ascontiguousarray(xs.reshape(NT, 128, 4).transpose(1, 0, 2))
        in_maps.append(m)
    return in_maps


_NC_CACHE = {}


def get_graph():
    if "nc" not in _NC_CACHE:
        _NC_CACHE["nc"] = build_graph()
    return _NC_CACHE["nc"]


def kernel(**inputs) -> np.ndarray:
    from concourse.bass_utils import run_bass_kernel_spmd

    nc = get_graph()
    in_maps = prep_inputs(inputs)
    res = run_bass_kernel_spmd(nc, in_maps, core_ids=list(range(NCORES)))
    return np.concatenate(
        [np.asarray(res.results[i]["out"], np.float32) for i in range(NCORES)], axis=0
    )
